# revision 24
# baseline (speedup 1.0000x reference)
"""Causal self-attention on 8 TRN2 NeuronCores.

Problem: x[2,2048,1024], wq/wk/wv/wo[1024,1024] (nn.Linear convention,
out = y @ W.T), H=16 heads, D=64, causal softmax, f32.

Sharding: tensor-parallel over heads x data-parallel over batch.
Core i handles batch b=i//4 and head group g=i%4 (4 heads each).
wq/wk/wv are split row-wise (output-feature) per head group; wo is
split column-wise; each core returns a partial output projection
out_partial[b] and the host sums the 4 partials per batch.

Key perf insight (from HW traces): the TRN2 PE clock (HAM state)
follows the *stationary-operand width* of recent matmuls -- windows
dominated by K=64 stationary tiles run at half clock (k=4) no matter
how dense the schedule is, while K=128 windows run at 2.4 GHz (k=8).
Per-head attention (D=64 contraction, 64-row V blocks) is therefore
stuck at half clock.  This kernel instead processes *head pairs*
through block-diagonal stationaries so every attention matmul is a
full [128,128]:
  scores^T: stat = blockdiag(kT_h0[64d,64keys], kT_h1) in bf16,
            moving = qT_pair[128d, q] -> both heads' scores for a
            64-key block in one matmul;
  PV:       stat = blockdiag(v_h0[64keys,64f], v_h1) -> both heads'
            unnormalized outputs stacked [128f, q];
  row sums: ones-block stationary [128,128] (2 live columns, rest
            zero-padded to keep the array full) -> sums in psum rows
            0:2.
Softmax exp runs on ScalarE in wide strokes; causal masking via
gpsimd affine_select on the bf16 exp output; the softmax 1/sum rides
a DRAM-transpose roundtrip so DVE reciprocal sees it 128-wide, with
the normalize muls deferred several key-blocks so the in-order DVE
stream never waits on the DMA chain.  Dummy warmup matmuls ramp the
PE clock while x streams in.
"""

import sys

for _p in ("/opt/trn_rl_repo", "/root/.axon_site"):
    if _p not in sys.path:
        sys.path.insert(0, _p)

import numpy as np

import concourse.bass as bass
import concourse.mybir as mybir
import concourse.tile as tile
from concourse import bacc
from concourse.bass_utils import run_bass_kernel_spmd

B, T, C, H = 2, 2048, 1024, 16
DH = C // H            # 64 head dim
HG = 4                 # heads per core
GW = HG * DH           # 256 features per head group
NB = T // 128          # 16 key chunks
NKB = T // 64          # 32 key half-blocks
NS = T // 512          # 4 query spans
KC = C // 128          # 8 contraction chunks over C
SCALE = 1.0 / float(np.sqrt(DH))
N_CORES = 8
N_WARM = 40            # dummy PE warmup matmuls

F32 = mybir.dt.float32
F32R = mybir.dt.float32r
BF16 = mybir.dt.bfloat16
EXP = mybir.ActivationFunctionType.Exp
COPY = mybir.ActivationFunctionType.Copy


def build_nc():
    nc = bacc.Bacc("TRN2", target_bir_lowering=False, debug=False,
                   num_devices=N_CORES)
    xT = nc.declare_dram_parameter("xT", [C, T], F32R, isOutput=False)
    wqT = nc.declare_dram_parameter("wqT", [C, GW], F32R, isOutput=False)
    wkT = nc.declare_dram_parameter("wkT", [C, GW], F32R, isOutput=False)
    wvT = nc.declare_dram_parameter("wvT", [C, GW], F32R, isOutput=False)
    woT = nc.declare_dram_parameter("woT", [GW, C], F32R, isOutput=False)
    outT = nc.declare_dram_parameter("outT", [C, T], F32, isOutput=True)
    wscr = nc.dram_tensor("w_scratch", [128, 4], F32)
    s_dram = nc.dram_tensor("s_scratch", [2, NS, 1024], F32)
    r_dram = nc.dram_tensor("r_scratch", [2, NS, 1024], F32)

    with tile.TileContext(nc) as tc:
        with tc.tile_pool(name="pers", bufs=1) as pers, \
             tc.tile_pool(name="pts", bufs=3) as pts, \
             tc.tile_pool(name="nrm", bufs=2) as nrm, \
             tc.tile_pool(name="ost", bufs=4) as ost:

            # ---- persistent tiles ----
            wk_t = [pers.tile([128, GW], F32R, tag=f"wk{i}", name=f"wk{i}")
                    for i in range(KC)]
            wq_t = [pers.tile([128, GW], F32R, tag=f"wq{i}", name=f"wq{i}")
                    for i in range(KC)]
            wv_t = [pers.tile([128, GW], F32R, tag=f"wv{i}", name=f"wv{i}")
                    for i in range(KC)]
            wo_t = [pers.tile([128, C], F32R, tag=f"wo{j}", name=f"wo{j}")
                    for j in range(2)]
            # per-pair tensors: qT (moving, bf16), blockdiag kT (stationary,
            # bf16, 128 cols per 64-key block), y (f32r)
            qtp = [pers.tile([128, T], BF16, tag=f"qT{m}", name=f"qT{m}")
                   for m in range(2)]
            ktp = [pers.tile([128, 2 * T], BF16, tag=f"kT{m}", name=f"kT{m}")
                   for m in range(2)]
            yts = [pers.tile([128, T], F32R, tag=f"yT{m}", name=f"yT{m}")
                   for m in range(2)]
            # blockdiag V per (pair, 64-key block)
            vbd = [[pers.tile([128, 128], BF16, tag=f"V{m}_{kb}",
                              name=f"V{m}_{kb}") for kb in range(NKB)]
                   for m in range(2)]
            onesbd = pers.tile([128, 128], BF16, tag="onesbd", name="onesbd")
            dumx = pers.tile([128, 512], BF16, tag="dumx", name="dumx")
            dsb = pers.tile([128, 4], F32, tag="dsb", name="dsb")

            # zero the blockdiag pads once, Pool is idle at start
            nc.gpsimd.memset(dumx, 0.0)
            for m in range(2):
                nc.gpsimd.memset(ktp[m], 0.0)
                for kb in range(NKB):
                    nc.gpsimd.memset(vbd[m][kb], 0.0)
            nc.gpsimd.memset(onesbd, 0.0)
            nc.scalar.activation(
                out=onesbd[0:64, 0:1],
                in_=nc.const_aps.tensor(1.0, [64, 1]), func=COPY)
            nc.scalar.activation(
                out=onesbd[64:128, 1:2],
                in_=nc.const_aps.tensor(1.0, [64, 1]), func=COPY)
            # causal mask for the diagonal 64-col strip of a blockdiag
            # pt block: both 64-row halves keep col >= (slice-local row)
            trim2 = pers.tile([128, 64], BF16, tag="trim2", name="trim2")
            nc.gpsimd.memset(trim2, 1.0)
            for hf in range(2):
                nc.gpsimd.affine_select(
                    out=trim2[hf * 64:(hf + 1) * 64, :],
                    in_=trim2[hf * 64:(hf + 1) * 64, :],
                    compare_op=mybir.AluOpType.is_ge,
                    fill=0.0, base=0, pattern=[[1, 64]],
                    channel_multiplier=-1)

            # ============ phase 1: warmup, loads, projections ============
            with tc.tile_pool(name="pp", bufs=4, space="PSUM") as pp, \
                 tc.tile_pool(name="xtp", bufs=1) as xtp:
                # PE warmup: dense full-width matmuls from t~0 while x loads
                wps = pp.tile([128, 512], F32, tag="pp", name="wps")
                for i in range(N_WARM):
                    nc.tensor.matmul(wps, dumx[:, 0:128], dumx,
                                     start=(i == 0), stop=(i == N_WARM - 1))
                nc.vector.tensor_copy(out=dsb, in_=wps[:, 0:4])
                nc.sync.dma_start(out=wscr[:, :], in_=dsb)

                xts = [xtp.tile([128, T], F32R, tag=f"xT{i}", name=f"xT{i}")
                       for i in range(KC)]
                for i in range(KC):
                    nc.sync.dma_start(out=wk_t[i],
                                      in_=wkT[i * 128:(i + 1) * 128, :])
                for i in range(KC):
                    nc.sync.dma_start(
                        out=xts[i][:, 0:1024],
                        in_=xT[i * 128:(i + 1) * 128, 0:1024])
                for i in range(KC):
                    nc.sync.dma_start(out=wq_t[i],
                                      in_=wqT[i * 128:(i + 1) * 128, :])
                for i in range(KC):
                    nc.sync.dma_start(out=wv_t[i],
                                      in_=wvT[i * 128:(i + 1) * 128, :])
                for i in range(KC):
                    nc.sync.dma_start(
                        out=xts[i][:, 1024:2048],
                        in_=xT[i * 128:(i + 1) * 128, 1024:2048])
                for j in range(2):
                    nc.sync.dma_start(out=wo_t[j],
                                      in_=woT[j * 128:(j + 1) * 128, :])

                def kq_span(s):
                    for m in range(2):
                        # K chunk -> blockdiag ktp quadrants (DVE, casts)
                        ps = pp.tile([128, 512], F32, tag="pp", name="kps")
                        for k in range(KC):
                            nc.tensor.matmul(
                                ps, wk_t[k][:, m * 128:(m + 1) * 128],
                                xts[k][:, s * 512:(s + 1) * 512],
                                start=(k == 0), stop=(k == KC - 1))
                        kv = ktp[m].rearrange("p (kb c) -> p kb c", c=128)
                        nc.vector.tensor_copy(
                            out=kv[0:64, 8 * s:8 * s + 8, 0:64],
                            in_=ps[0:64, :].rearrange(
                                "p (kb c) -> p kb c", c=64))
                        nc.vector.tensor_copy(
                            out=kv[64:128, 8 * s:8 * s + 8, 64:128],
                            in_=ps[64:128, :].rearrange(
                                "p (kb c) -> p kb c", c=64))
                        # Q chunk -> qtp (ScalarE, casts)
                        ps2 = pp.tile([128, 512], F32, tag="pp", name="qps")
                        for k in range(KC):
                            nc.tensor.matmul(
                                ps2, wq_t[k][:, m * 128:(m + 1) * 128],
                                xts[k][:, s * 512:(s + 1) * 512],
                                start=(k == 0), stop=(k == KC - 1))
                        nc.scalar.activation(
                            out=qtp[m][:, s * 512:(s + 1) * 512], in_=ps2,
                            func=COPY)

                def v_tb(tb):
                    # V -> blockdiag vbd quadrants
                    vps = pp.tile([128, GW], F32, tag="pp", name="vps")
                    for k in range(KC):
                        nc.tensor.matmul(
                            vps, xts[k][:, tb * 128:(tb + 1) * 128], wv_t[k],
                            start=(k == 0), stop=(k == KC - 1))
                    for m in range(2):
                        for hf in range(2):
                            vt = vbd[m][2 * tb + hf]
                            rows = slice(hf * 64, (hf + 1) * 64)
                            nc.vector.tensor_copy(
                                out=vt[0:64, 0:64],
                                in_=vps[rows, m * 128:m * 128 + 64])
                            nc.vector.tensor_copy(
                                out=vt[64:128, 64:128],
                                in_=vps[rows, m * 128 + 64:m * 128 + 128])

                # order: spans 0,1 + V0-7 run off x-half0 while half1 lands
                kq_span(0)
                kq_span(1)
                for tb in range(8):
                    v_tb(tb)
                kq_span(2)
                kq_span(3)
                for tb in range(8, 16):
                    v_tb(tb)

            # ============ phase 2: attention, one head-pair at a time ====
            pending = []   # deferred [delay, closure] normalize muls

            def finalize(m, sg, pv, sm, yt):
                yv = nrm.tile([128, 512], F32, tag="yv", name="yv")
                nc.vector.tensor_copy(out=yv, in_=pv)
                sv = nrm.tile([2, 512], F32, tag="sv", name="sv")
                nc.vector.tensor_copy(out=sv, in_=sm[0:2, :])
                nc.sync.dma_start(out=s_dram[m, sg, :], in_=sv)
                st = nrm.tile([128, 8], F32, tag="st", name="st")
                nc.sync.dma_start(
                    out=st,
                    in_=s_dram[m, sg, :].rearrange("(c p) -> p c", p=128))
                rts = nrm.tile([128, 8], F32, tag="rts", name="rts")
                nc.vector.reciprocal(out=rts, in_=st)
                nc.sync.dma_start(
                    out=r_dram[m, sg, :].rearrange("(c p) -> p c", p=128),
                    in_=rts)
                rb = nrm.tile([128, 512], F32, tag="rb", name="rb")
                for hf in range(2):
                    rsl = r_dram[m, sg, hf * 512:(hf + 1) * 512]
                    nc.sync.dma_start(
                        out=rb[hf * 64:(hf + 1) * 64, :],
                        in_=bass.AP(tensor=rsl.tensor, offset=rsl.offset,
                                    ap=[[0, 64]] + list(rsl.ap)))

                def _mul():
                    nc.vector.tensor_mul(
                        out=yt[:, sg * 512:(sg + 1) * 512], in0=yv, in1=rb)
                pending.append([6, _mul])

            with tc.tile_pool(name="mgs", bufs=2, space="PSUM") as mgs, \
                 tc.tile_pool(name="pvs", bufs=1, space="PSUM") as pvs, \
                 tc.tile_pool(name="sms", bufs=1, space="PSUM") as sms:
                for m in range(2):
                    for uh in range(2):
                        qt, kt, yt = qtp[m], ktp[m], yts[m]
                        q0 = uh * 1024
                        spans = (2 * uh, 2 * uh + 1)
                        pv_t = [pvs.tile([128, 512], F32, tag=f"pv{sp}",
                                         name=f"pv{sp}") for sp in range(2)]
                        sm_t = [sms.tile([128, 512], F32, tag=f"sm{sp}",
                                         name=f"sm{sp}") for sp in range(2)]
                        for kb in range(16 if uh == 0 else NKB):
                            dcol = kb * 64 - q0
                            lo = max(dcol, 0)
                            mg = mgs.tile([128, 1024], F32, tag="mg",
                                          name="mg")
                            for sp in range(2):
                                a = max(lo, sp * 512)
                                b = (sp + 1) * 512
                                if a < b:
                                    nc.tensor.matmul(
                                        mg[:, a:b],
                                        kt[:, kb * 128:(kb + 1) * 128],
                                        qt[:, q0 + a:q0 + b],
                                        start=True, stop=True)
                            pt = pts.tile([128, 1024], BF16, tag="pt",
                                          name="pt")
                            nc.scalar.activation(
                                out=pt[:, lo:1024], in_=mg[:, lo:1024],
                                func=EXP, scale=SCALE)
                            if dcol >= 0:
                                nc.vector.tensor_mul(
                                    out=pt[:, dcol:dcol + 64],
                                    in0=pt[:, dcol:dcol + 64], in1=trim2)
                            for ent in pending:
                                ent[0] -= 1
                            while pending and pending[0][0] <= 0:
                                pending.pop(0)[1]()
                            for sp in range(2):
                                sg = spans[sp]
                                last = 8 * sg + 7
                                a = max(lo, sp * 512)
                                b = (sp + 1) * 512
                                if kb <= last and a < b:
                                    nc.tensor.matmul(
                                        pv_t[sp][:, a - sp * 512:512],
                                        vbd[m][kb], pt[:, a:b],
                                        start=(kb == 0), stop=(kb == last))
                                    nc.tensor.matmul(
                                        sm_t[sp][:, a - sp * 512:512],
                                        onesbd, pt[:, a:b],
                                        start=(kb == 0), stop=(kb == last))
                                if kb == last:
                                    finalize(m, sg, pv_t[sp], sm_t[sp], yt)
            while pending:
                pending.pop(0)[1]()

            # ============ phase 3: output projection ============
            with tc.tile_pool(name="ops", bufs=4, space="PSUM") as ops:
                for s in range(NS):
                    for mo in range(8):
                        op = ops.tile([128, 512], F32, tag="op", name="op")
                        for j in range(2):
                            nc.tensor.matmul(
                                op, wo_t[j][:, mo * 128:(mo + 1) * 128],
                                yts[j][:, s * 512:(s + 1) * 512],
                                start=(j == 0), stop=(j == 1))
                        ot = ost.tile([128, 512], F32, tag="ot", name="ot")
                        nc.vector.tensor_copy(out=ot, in_=op)
                        nc.sync.dma_start(
                            out=outT[mo * 128:(mo + 1) * 128,
                                     s * 512:(s + 1) * 512],
                            in_=ot)
    nc.compile()
    return nc


_NC_CACHE = None


def _get_nc():
    global _NC_CACHE
    if _NC_CACHE is None:
        _NC_CACHE = build_nc()
    return _NC_CACHE


def make_in_maps(x, wq, wk, wv, wo):
    x = np.asarray(x, dtype=np.float32)
    wq = np.asarray(wq, dtype=np.float32)
    wk = np.asarray(wk, dtype=np.float32)
    wv = np.asarray(wv, dtype=np.float32)
    wo = np.asarray(wo, dtype=np.float32)
    in_maps = []
    for core in range(N_CORES):
        b, g = core // HG, core % HG
        rows = slice(g * GW, (g + 1) * GW)
        in_maps.append({
            "xT": np.ascontiguousarray(x[b].T),
            "wqT": np.ascontiguousarray(wq[rows, :].T),
            "wkT": np.ascontiguousarray(wk[rows, :].T),
            "wvT": np.ascontiguousarray(wv[rows, :].T),
            "woT": np.ascontiguousarray(wo[:, rows].T),
        })
    return in_maps


def run(x, wq, wk, wv, wo, trace=False, tmpdir=None):
    nc = _get_nc()
    in_maps = make_in_maps(x, wq, wk, wv, wo)
    res = run_bass_kernel_spmd(nc, in_maps, core_ids=list(range(N_CORES)),
                               trace=trace, tmpdir=tmpdir)
    out = np.zeros((B, T, C), dtype=np.float32)
    for core in range(N_CORES):
        out[core // HG] += res.results[core]["outT"].T
    return out, res


def kernel(x, wq, wk, wv, wo):
    out, _ = run(x, wq, wk, wv, wo)
    return out


# revision 25
# speedup vs baseline: 1.0104x; 1.0104x over previous
"""Causal self-attention on 8 TRN2 NeuronCores.

Problem: x[2,2048,1024], wq/wk/wv/wo[1024,1024] (nn.Linear convention,
out = y @ W.T), H=16 heads, D=64, causal softmax, f32.

Sharding: tensor-parallel over heads x data-parallel over batch.
Core i handles batch b=i//4 and head group g=i%4 (4 heads each).
wq/wk/wv are split row-wise (output-feature) per head group; wo is
split column-wise; each core returns a partial output projection
out_partial[b] and the host sums the 4 partials per batch.

Key perf insight (from HW traces): the TRN2 PE clock (HAM state)
follows the *stationary-operand width* of recent matmuls -- windows
dominated by K=64 stationary tiles run at half clock (k=4) no matter
how dense the schedule is, while K=128 windows run at 2.4 GHz (k=8).
Per-head attention (D=64 contraction, 64-row V blocks) is therefore
stuck at half clock.  This kernel instead processes *head pairs*
through block-diagonal stationaries so every attention matmul is a
full [128,128]:
  scores^T: stat = blockdiag(kT_h0[64d,64keys], kT_h1) in bf16,
            moving = qT_pair[128d, q] -> both heads' scores for a
            64-key block in one matmul;
  PV:       stat = blockdiag(v_h0[64keys,64f], v_h1) -> both heads'
            unnormalized outputs stacked [128f, q];
  row sums: ones-block stationary [128,128] (2 live columns, rest
            zero-padded to keep the array full) -> sums in psum rows
            0:2.
Softmax exp runs on ScalarE in wide strokes; causal masking via
gpsimd affine_select on the bf16 exp output; the softmax 1/sum rides
a DRAM-transpose roundtrip so DVE reciprocal sees it 128-wide, with
the normalize muls deferred several key-blocks so the in-order DVE
stream never waits on the DMA chain.  Dummy warmup matmuls ramp the
PE clock while x streams in.
"""

import sys

for _p in ("/opt/trn_rl_repo", "/root/.axon_site"):
    if _p not in sys.path:
        sys.path.insert(0, _p)

import numpy as np

import concourse.bass as bass
import concourse.mybir as mybir
import concourse.tile as tile
from concourse import bacc
from concourse.bass_utils import run_bass_kernel_spmd

B, T, C, H = 2, 2048, 1024, 16
DH = C // H            # 64 head dim
HG = 4                 # heads per core
GW = HG * DH           # 256 features per head group
NB = T // 128          # 16 key chunks
NKB = T // 64          # 32 key half-blocks
NS = T // 512          # 4 query spans
KC = C // 128          # 8 contraction chunks over C
SCALE = 1.0 / float(np.sqrt(DH))
N_CORES = 8
N_WARM = 40            # dummy PE warmup matmuls

F32 = mybir.dt.float32
F32R = mybir.dt.float32r
BF16 = mybir.dt.bfloat16
EXP = mybir.ActivationFunctionType.Exp
COPY = mybir.ActivationFunctionType.Copy


def build_nc():
    nc = bacc.Bacc("TRN2", target_bir_lowering=False, debug=False,
                   num_devices=N_CORES)
    xT = nc.declare_dram_parameter("xT", [C, T], F32R, isOutput=False)
    wqT = nc.declare_dram_parameter("wqT", [C, GW], F32R, isOutput=False)
    wkT = nc.declare_dram_parameter("wkT", [C, GW], F32R, isOutput=False)
    wvT = nc.declare_dram_parameter("wvT", [C, GW], F32R, isOutput=False)
    woT = nc.declare_dram_parameter("woT", [GW, C], F32R, isOutput=False)
    outT = nc.declare_dram_parameter("outT", [C, T], F32, isOutput=True)
    wscr = nc.dram_tensor("w_scratch", [128, 4], F32)
    s_dram = nc.dram_tensor("s_scratch", [2, NS, 1024], F32)
    r_dram = nc.dram_tensor("r_scratch", [2, NS, 1024], F32)

    with tile.TileContext(nc) as tc:
        with tc.tile_pool(name="pers", bufs=1) as pers, \
             tc.tile_pool(name="pts", bufs=3) as pts, \
             tc.tile_pool(name="nrm", bufs=2) as nrm, \
             tc.tile_pool(name="ost", bufs=4) as ost:

            # ---- persistent tiles ----
            wk_t = [pers.tile([128, GW], F32R, tag=f"wk{i}", name=f"wk{i}")
                    for i in range(KC)]
            wq_t = [pers.tile([128, GW], F32R, tag=f"wq{i}", name=f"wq{i}")
                    for i in range(KC)]
            wv_t = [pers.tile([128, GW], F32R, tag=f"wv{i}", name=f"wv{i}")
                    for i in range(KC)]
            wo_t = [pers.tile([128, C], F32R, tag=f"wo{j}", name=f"wo{j}")
                    for j in range(2)]
            # per-pair tensors: qT (moving, bf16), blockdiag kT (stationary,
            # bf16, 128 cols per 64-key block), y (f32r)
            qtp = [pers.tile([128, T], BF16, tag=f"qT{m}", name=f"qT{m}")
                   for m in range(2)]
            ktp = [pers.tile([128, 2 * T], BF16, tag=f"kT{m}", name=f"kT{m}")
                   for m in range(2)]
            yts = [pers.tile([128, T], F32R, tag=f"yT{m}", name=f"yT{m}")
                   for m in range(2)]
            # blockdiag V per (pair, 64-key block)
            vbd = [[pers.tile([128, 128], BF16, tag=f"V{m}_{kb}",
                              name=f"V{m}_{kb}") for kb in range(NKB)]
                   for m in range(2)]
            onesbd = pers.tile([128, 128], BF16, tag="onesbd", name="onesbd")
            dumx = pers.tile([128, 512], BF16, tag="dumx", name="dumx")
            dsb = pers.tile([128, 4], F32, tag="dsb", name="dsb")

            # zero the blockdiag pads once, Pool is idle at start
            nc.gpsimd.memset(dumx, 0.0)
            for m in range(2):
                nc.gpsimd.memset(ktp[m], 0.0)
                for kb in range(NKB):
                    nc.gpsimd.memset(vbd[m][kb], 0.0)
            nc.gpsimd.memset(onesbd, 0.0)
            nc.scalar.activation(
                out=onesbd[0:64, 0:1],
                in_=nc.const_aps.tensor(1.0, [64, 1]), func=COPY)
            nc.scalar.activation(
                out=onesbd[64:128, 1:2],
                in_=nc.const_aps.tensor(1.0, [64, 1]), func=COPY)
            # causal mask for the diagonal 64-col strip of a blockdiag
            # pt block: both 64-row halves keep col >= (slice-local row)
            trim2 = pers.tile([128, 64], BF16, tag="trim2", name="trim2")
            nc.gpsimd.memset(trim2, 1.0)
            for hf in range(2):
                nc.gpsimd.affine_select(
                    out=trim2[hf * 64:(hf + 1) * 64, :],
                    in_=trim2[hf * 64:(hf + 1) * 64, :],
                    compare_op=mybir.AluOpType.is_ge,
                    fill=0.0, base=0, pattern=[[1, 64]],
                    channel_multiplier=-1)

            # ============ phase 1: warmup, loads, projections ============
            with tc.tile_pool(name="pp", bufs=4, space="PSUM") as pp, \
                 tc.tile_pool(name="xtp", bufs=1) as xtp:
                # PE warmup: dense full-width matmuls from t~0 while x loads
                wps = pp.tile([128, 512], F32, tag="pp", name="wps")
                for i in range(N_WARM):
                    nc.tensor.matmul(wps, dumx[:, 0:128], dumx,
                                     start=(i == 0), stop=(i == N_WARM - 1))
                nc.vector.tensor_copy(out=dsb, in_=wps[:, 0:4])
                nc.sync.dma_start(out=wscr[:, :], in_=dsb)

                xts = [xtp.tile([128, T], F32R, tag=f"xT{i}", name=f"xT{i}")
                       for i in range(KC)]
                for i in range(KC):
                    nc.sync.dma_start(out=wk_t[i],
                                      in_=wkT[i * 128:(i + 1) * 128, :])
                for i in range(KC):
                    nc.sync.dma_start(
                        out=xts[i][:, 0:1024],
                        in_=xT[i * 128:(i + 1) * 128, 0:1024])
                for i in range(KC):
                    nc.sync.dma_start(out=wq_t[i],
                                      in_=wqT[i * 128:(i + 1) * 128, :])
                for i in range(KC):
                    nc.sync.dma_start(out=wv_t[i],
                                      in_=wvT[i * 128:(i + 1) * 128, :])
                for i in range(KC):
                    nc.sync.dma_start(
                        out=xts[i][:, 1024:2048],
                        in_=xT[i * 128:(i + 1) * 128, 1024:2048])
                for j in range(2):
                    nc.sync.dma_start(out=wo_t[j],
                                      in_=woT[j * 128:(j + 1) * 128, :])

                def kq_span(s):
                    for m in range(2):
                        # K chunk -> blockdiag ktp quadrants (DVE, casts)
                        ps = pp.tile([128, 512], F32, tag="pp", name="kps")
                        for k in range(KC):
                            nc.tensor.matmul(
                                ps, wk_t[k][:, m * 128:(m + 1) * 128],
                                xts[k][:, s * 512:(s + 1) * 512],
                                start=(k == 0), stop=(k == KC - 1))
                        kv = ktp[m].rearrange("p (kb c) -> p kb c", c=128)
                        nc.vector.tensor_copy(
                            out=kv[0:64, 8 * s:8 * s + 8, 0:64],
                            in_=ps[0:64, :].rearrange(
                                "p (kb c) -> p kb c", c=64))
                        nc.vector.tensor_copy(
                            out=kv[64:128, 8 * s:8 * s + 8, 64:128],
                            in_=ps[64:128, :].rearrange(
                                "p (kb c) -> p kb c", c=64))
                        # Q chunk -> qtp (ScalarE, casts)
                        ps2 = pp.tile([128, 512], F32, tag="pp", name="qps")
                        for k in range(KC):
                            nc.tensor.matmul(
                                ps2, wq_t[k][:, m * 128:(m + 1) * 128],
                                xts[k][:, s * 512:(s + 1) * 512],
                                start=(k == 0), stop=(k == KC - 1))
                        nc.scalar.activation(
                            out=qtp[m][:, s * 512:(s + 1) * 512], in_=ps2,
                            func=COPY)

                def v_tb(tb):
                    # V -> blockdiag vbd quadrants
                    vps = pp.tile([128, GW], F32, tag="pp", name="vps")
                    for k in range(KC):
                        nc.tensor.matmul(
                            vps, xts[k][:, tb * 128:(tb + 1) * 128], wv_t[k],
                            start=(k == 0), stop=(k == KC - 1))
                    for m in range(2):
                        for hf in range(2):
                            vt = vbd[m][2 * tb + hf]
                            rows = slice(hf * 64, (hf + 1) * 64)
                            nc.vector.tensor_copy(
                                out=vt[0:64, 0:64],
                                in_=vps[rows, m * 128:m * 128 + 64])
                            nc.vector.tensor_copy(
                                out=vt[64:128, 64:128],
                                in_=vps[rows, m * 128 + 64:m * 128 + 128])

                # order: spans 0,1 + V0-7 run off x-half0 while half1 lands
                kq_span(0)
                kq_span(1)
                for tb in range(8):
                    v_tb(tb)
                kq_span(2)
                kq_span(3)
                for tb in range(8, 16):
                    v_tb(tb)

            # ============ phase 2: attention, one head-pair at a time ====
            pending = []   # deferred [delay, closure] normalize muls

            def finalize(m, sg, pv, sm, yt):
                # stage A (now): copy out of PSUM (frees the banks), kick
                # off the sums DRAM-transpose; stage B (+3 kbs): DVE
                # reciprocal once the transposed load surely landed, kick
                # off the broadcast; stage C (+6 kbs): normalize mul.
                # Staging keeps every DVE op wait-free so the in-order DVE
                # stream never stalls the mask muls that gate PV.
                yv = nrm.tile([128, 512], F32, tag="yv", name="yv")
                nc.vector.tensor_copy(out=yv, in_=pv)
                sv = nrm.tile([2, 512], F32, tag="sv", name="sv")
                nc.vector.tensor_copy(out=sv, in_=sm[0:2, :])
                nc.gpsimd.dma_start(out=s_dram[m, sg, :], in_=sv)
                st = nrm.tile([128, 8], F32, tag="st", name="st")
                nc.gpsimd.dma_start(
                    out=st,
                    in_=s_dram[m, sg, :].rearrange("(c p) -> p c", p=128))
                rb = nrm.tile([128, 512], F32, tag="rb", name="rb")

                def _recip_bcast():
                    rts = nrm.tile([128, 8], F32, tag="rts", name="rts")
                    nc.vector.reciprocal(out=rts, in_=st)
                    nc.gpsimd.dma_start(
                        out=r_dram[m, sg, :].rearrange("(c p) -> p c", p=128),
                        in_=rts)
                    for hf in range(2):
                        rsl = r_dram[m, sg, hf * 512:(hf + 1) * 512]
                        nc.gpsimd.dma_start(
                            out=rb[hf * 64:(hf + 1) * 64, :],
                            in_=bass.AP(tensor=rsl.tensor, offset=rsl.offset,
                                        ap=[[0, 64]] + list(rsl.ap)))

                def _mul():
                    nc.vector.tensor_mul(
                        out=yt[:, sg * 512:(sg + 1) * 512], in0=yv, in1=rb)
                pending.append([3, _recip_bcast])
                pending.append([6, _mul])

            with tc.tile_pool(name="mgs", bufs=2, space="PSUM") as mgs, \
                 tc.tile_pool(name="pvs", bufs=1, space="PSUM") as pvs, \
                 tc.tile_pool(name="sms", bufs=1, space="PSUM") as sms:
                for m in range(2):
                    for uh in range(2):
                        qt, kt, yt = qtp[m], ktp[m], yts[m]
                        q0 = uh * 1024
                        spans = (2 * uh, 2 * uh + 1)
                        pv_t = [pvs.tile([128, 512], F32, tag=f"pv{sp}",
                                         name=f"pv{sp}") for sp in range(2)]
                        sm_t = [sms.tile([128, 512], F32, tag=f"sm{sp}",
                                         name=f"sm{sp}") for sp in range(2)]
                        for kb in range(16 if uh == 0 else NKB):
                            dcol = kb * 64 - q0
                            lo = max(dcol, 0)
                            mg = mgs.tile([128, 1024], F32, tag="mg",
                                          name="mg")
                            for sp in range(2):
                                a = max(lo, sp * 512)
                                b = (sp + 1) * 512
                                if a < b:
                                    nc.tensor.matmul(
                                        mg[:, a:b],
                                        kt[:, kb * 128:(kb + 1) * 128],
                                        qt[:, q0 + a:q0 + b],
                                        start=True, stop=True)
                            pt = pts.tile([128, 1024], BF16, tag="pt",
                                          name="pt")
                            nc.scalar.activation(
                                out=pt[:, lo:1024], in_=mg[:, lo:1024],
                                func=EXP, scale=SCALE)
                            if dcol >= 0:
                                nc.vector.tensor_mul(
                                    out=pt[:, dcol:dcol + 64],
                                    in0=pt[:, dcol:dcol + 64], in1=trim2)
                            for ent in pending:
                                ent[0] -= 1
                            while pending and pending[0][0] <= 0:
                                pending.pop(0)[1]()
                            for sp in range(2):
                                sg = spans[sp]
                                last = 8 * sg + 7
                                a = max(lo, sp * 512)
                                b = (sp + 1) * 512
                                if kb <= last and a < b:
                                    nc.tensor.matmul(
                                        pv_t[sp][:, a - sp * 512:512],
                                        vbd[m][kb], pt[:, a:b],
                                        start=(kb == 0), stop=(kb == last))
                                    nc.tensor.matmul(
                                        sm_t[sp][:, a - sp * 512:512],
                                        onesbd, pt[:, a:b],
                                        start=(kb == 0), stop=(kb == last))
                                if kb == last:
                                    finalize(m, sg, pv_t[sp], sm_t[sp], yt)
            while pending:
                pending.pop(0)[1]()

            # ============ phase 3: output projection ============
            with tc.tile_pool(name="ops", bufs=4, space="PSUM") as ops:
                for s in range(NS):
                    for mo in range(8):
                        op = ops.tile([128, 512], F32, tag="op", name="op")
                        for j in range(2):
                            nc.tensor.matmul(
                                op, wo_t[j][:, mo * 128:(mo + 1) * 128],
                                yts[j][:, s * 512:(s + 1) * 512],
                                start=(j == 0), stop=(j == 1))
                        ot = ost.tile([128, 512], F32, tag="ot", name="ot")
                        nc.vector.tensor_copy(out=ot, in_=op)
                        nc.sync.dma_start(
                            out=outT[mo * 128:(mo + 1) * 128,
                                     s * 512:(s + 1) * 512],
                            in_=ot)
    nc.compile()
    return nc


_NC_CACHE = None


def _get_nc():
    global _NC_CACHE
    if _NC_CACHE is None:
        _NC_CACHE = build_nc()
    return _NC_CACHE


def make_in_maps(x, wq, wk, wv, wo):
    x = np.asarray(x, dtype=np.float32)
    wq = np.asarray(wq, dtype=np.float32)
    wk = np.asarray(wk, dtype=np.float32)
    wv = np.asarray(wv, dtype=np.float32)
    wo = np.asarray(wo, dtype=np.float32)
    in_maps = []
    for core in range(N_CORES):
        b, g = core // HG, core % HG
        rows = slice(g * GW, (g + 1) * GW)
        in_maps.append({
            "xT": np.ascontiguousarray(x[b].T),
            "wqT": np.ascontiguousarray(wq[rows, :].T),
            "wkT": np.ascontiguousarray(wk[rows, :].T),
            "wvT": np.ascontiguousarray(wv[rows, :].T),
            "woT": np.ascontiguousarray(wo[:, rows].T),
        })
    return in_maps


def run(x, wq, wk, wv, wo, trace=False, tmpdir=None):
    nc = _get_nc()
    in_maps = make_in_maps(x, wq, wk, wv, wo)
    res = run_bass_kernel_spmd(nc, in_maps, core_ids=list(range(N_CORES)),
                               trace=trace, tmpdir=tmpdir)
    out = np.zeros((B, T, C), dtype=np.float32)
    for core in range(N_CORES):
        out[core // HG] += res.results[core]["outT"].T
    return out, res


def kernel(x, wq, wk, wv, wo):
    out, _ = run(x, wq, wk, wv, wo)
    return out
